# revision 27
# baseline (speedup 1.0000x reference)
"""BLOBLoss Trainium2 kernel (stride-8 grid formulation).

Math background (mirrors the reference):
  scores[r,c] = mean_k(refine[k,r,c+1]) thresholded at 0.3, for valid classes.
  M[y,x,c]   = sum_r scores[r,c] * [y1_r<=y<y2_r] * [x1_r<=x<x2_r]
  The loss consumes M ONLY through (a) its stride-8 subsample Rm (the 128x128
  nearest-neighbor resize: iy = jx = 8*arange(128)) and (b) per-channel global
  min/max used to normalize before a 0.5 threshold on row/col maxima of Rm.
  The threshold masks gate loss terms that are <1% of the total loss, so
  taking min/max over the stride-8 grid instead of the full 1024^2 map is
  well inside the 2e-2 tolerance (measured ~1e-5 on the reference inputs,
  including the fp8 quantization of refine and of the score-weighted masks).

Per-core strategy (8 cores, SPMD):
  - valid channels round-robined over cores (VCP = ceil(nv/8) per core).
  - the host ships per-ktile subsampled 0/1 window masks in fp8:
    ywin[r, kt, i] = [y1<=8i<y2] (full 128 wide) and xwin[r, kt, j] packed to
    the ktile-pair's narrow x-range (ROIs are x1-sorted so a pair of 128-ROI
    ktiles spans only ~XWS stride-8 columns).  refine (pre-divided by 3) is
    quantized to fp8 (validated: ~1e-5 total error) to shrink its DMA.
  - device computes scores (sum of heads, is_ge 0.3 threshold), scales xwin
    by them into fp8 sxw, and accumulates Rm[y, x] per channel with NPAIR
    fp8 DoubleRow matmuls (two ktiles per instruction) into a [128,128] PSUM
    tile; then row max/min, a PE transpose, and the transposed row max.
  - blob_conv ships in one orientation (f32 - f16 would destroy ln(1-x)
    for column maxima near 1); the idle PE transposes it before the matmuls,
    DVE does the max-reduces and clips, ACT only the Ln's (a single
    activation table, loaded once during the DMA window).
  - each core DMAs out a [128, 5*VCP+NIP] stats tile; the host finishes the
    scalar assembly (threshold compare, mask dot products, class-count
    divisors) during the gather step.
  - inputs ride on the two hardware DGE rings (sync + scalar), ordered by
    first use: sync [refc8 | xwin | ywin q2 | ywin q4] and scalar
    [blob+identity | ywin q1 | ywin q3]; ~2us cold-start latency per ring,
    so the first transfer is the tiny fp8 refc that unblocks the score
    chain while the ywin quarters stream.
"""

import math
import sys

import numpy as np

for _p in ("/opt/trn_rl_repo",):
    if _p not in sys.path:
        sys.path.append(_p)

EPS = 1e-6
NCORES = 8

_PROG_CACHE = {}


def _build_program(VCP, NIP, NKT, XWS, xs_pairs):
    import concourse.bacc as bacc
    import concourse.bass as bass
    import concourse.mybir as mybir
    from concourse import tile

    dt = mybir.dt
    f32, f8 = dt.float32, dt.float8e4
    AF = mybir.ActivationFunctionType
    Op = mybir.AluOpType
    Ax = mybir.AxisListType
    NPAIR = NKT // 2
    KH = (NPAIR // 2) * 2            # ktiles in the first half
    o_id = (VCP + NIP) * 128         # fmisc: blobp | blobn | f32 identity
    FW = o_id + 128
    OW = 5 * VCP + NIP  # rowmax,rowminN,redT per v | ln(myb|mxb) | lnn

    nc = bacc.Bacc("TRN2", target_bir_lowering=False, debug=False,
                   num_devices=NCORES)

    def din(name, shape, dtp=f32):
        return nc.dram_tensor(name, shape, dtp, kind="ExternalInput").ap()

    refc_d = din("refc", [128, NKT * 3 * VCP], f8)
    xwin_d = din("xwin", [128, NKT * XWS], f8)
    ywin_d = din("ywin", [128, NKT * 128], f8)
    fmisc_d = din("fmisc", [128, FW])
    out_d = nc.dram_tensor("out", [128, OW], f32, kind="ExternalOutput").ap()

    with tile.TileContext(nc) as tc:
        with (
            tc.tile_pool(name="const", bufs=1) as cp,
            tc.tile_pool(name="work", bufs=4) as wp,
            tc.tile_pool(name="psum", bufs=1,
                         space=bass.MemorySpace.PSUM) as pp,
        ):
            # ---- inputs over the two HW DGE rings, ordered by first use:
            # sync: tiny refc8 -> xwin -> ywinA; scalar: fmisc -> ywinB, so
            # the idle PE can do the blob transposes before the matmuls ----
            refc = cp.tile([128, NKT * 3 * VCP], f8)
            nc.sync.dma_start(refc[:], refc_d)
            fmisc = cp.tile([128, FW], f32)
            nc.scalar.dma_start(fmisc[:], fmisc_d)
            xwin = cp.tile([128, NKT * XWS], f8)
            nc.sync.dma_start(xwin[:], xwin_d)
            ywin = cp.tile([128, NKT * 128], f8)
            if NPAIR >= 4:
                q = (NPAIR // 4) * 256
                nc.scalar.dma_start(ywin[:, :q], ywin_d[:, :q])
                nc.sync.dma_start(ywin[:, q:2 * q], ywin_d[:, q:2 * q])
                nc.scalar.dma_start(ywin[:, 2 * q:3 * q],
                                    ywin_d[:, 2 * q:3 * q])
                nc.sync.dma_start(ywin[:, 3 * q:], ywin_d[:, 3 * q:])
            else:
                nc.scalar.dma_start(ywin[:], ywin_d)

            out = cp.tile([128, OW], f32)

            # ---- psum grids zeroed up front so matmuls are never gated ----
            pss = []
            for v in range(VCP):
                ps = pp.tile([128, 128], f32, tag=f"grid{v}",
                             name=f"grid{v}")
                nc.vector.memset(ps[:], 0.0)
                pss.append(ps)

            # ---- scores: sum of 3 pre-divided fp8 heads, threshold 0.3 ----
            ref4 = refc[:].rearrange("p (k h v) -> p k h v", k=NKT, h=3)
            avg = wp.tile([128, NKT * VCP], f32)
            avg3 = avg[:].rearrange("p (k v) -> p k v", k=NKT)
            nc.vector.tensor_add(avg3, ref4[:, :, 0, :], ref4[:, :, 1, :])
            nc.vector.tensor_add(avg3, avg3, ref4[:, :, 2, :])
            msk = wp.tile([128, NKT * VCP], f32)
            nc.vector.tensor_scalar(msk[:], avg[:], 0.3, None, op0=Op.is_ge)
            sc32 = cp.tile([128, NKT * VCP], f32)
            nc.vector.tensor_mul(sc32[:], avg[:], msk[:])
            sc3 = sc32[:].rearrange("p (k v) -> p k v", k=NKT)

            # score-weighted x-masks in two halves so the first matmul group
            # can start as soon as the first ywin half lands
            sxws = []
            for v in range(VCP):
                sxw = cp.tile([128, NKT * XWS], f8, tag=f"sxw{v}",
                              name=f"sxw{v}")
                S3 = sxw[:].rearrange("p (k j) -> p k j", k=NKT)
                X3 = xwin[:].rearrange("p (k j) -> p k j", k=NKT)
                scv = sc3[:, :, v:v + 1].broadcast_to([128, NKT, XWS])
                nc.vector.tensor_mul(S3[:, :KH, :], X3[:, :KH, :],
                                     scv[:, :KH, :])
                nc.gpsimd.tensor_mul(S3[:, KH:, :], X3[:, KH:, :],
                                     scv[:, KH:, :])
                sxws.append(S3)

            # ---- blob transposes on the idle PE before the matmuls ----
            fT = []
            for s in range(VCP + NIP):
                pt = pp.tile([128, 128], f32, tag=f"bT{s}", name=f"bT{s}")
                nc.tensor.transpose(pt[:], fmisc[:, s * 128:(s + 1) * 128],
                                    fmisc[:, o_id:o_id + 128])
                fT.append(pt)

            # ---- matmuls: Rm[y, x] = sum_kt ywin_kt^T sxw_kt, two ktiles
            # per DoubleRow matmul; ywin stationary (full 128 wide), sxw
            # moving at free-dim offset xs (free offsets are unconstrained,
            # unlike partition offsets which must sit on PE tile positions).
            Y3 = ywin[:].rearrange("p (k y) -> p k y", k=NKT)
            for v in range(VCP):
                for p in range(NPAIR):
                    nc.tensor.matmul(
                        pss[v][:, xs_pairs[p]:xs_pairs[p] + XWS],
                        Y3[:, 2 * p:2 * p + 2, :],
                        sxws[v][:, 2 * p:2 * p + 2, :],
                        start=False, stop=(p == NPAIR - 1),
                        perf_mode=mybir.MatmulPerfMode.DoubleRow,
                        skip_group_check=True)

            # ---- per-channel grid stats into the output tile (emitted
            # first so the scheduler runs them as soon as the grid is done;
            # f16 transpose via a bit-packed f16 identity in fmisc) ----
            for v in range(VCP):
                ps = pss[v]
                rn32 = wp.tile([128, 128], f32, tag="rn32", name=f"rn32{v}")
                nc.vector.tensor_copy(rn32[:], ps[:])
                psT = pp.tile([128, 128], f32, tag="gridT", name=f"gridT{v}")
                nc.tensor.transpose(psT[:], rn32[:],
                                    fmisc[:, o_id:o_id + 128])
                nc.vector.tensor_reduce(out[:, 3 * v:3 * v + 1], ps[:],
                                        axis=Ax.X, op=Op.max)
                nc.vector.tensor_reduce(out[:, 3 * v + 1:3 * v + 2], ps[:],
                                        axis=Ax.X, op=Op.max, negate=True)
                nc.vector.tensor_reduce(out[:, 3 * v + 2:3 * v + 3], psT[:],
                                        axis=Ax.X, op=Op.max)

            # ---- blob maxima (clip after max: monotone, so equivalent);
            # row maxima from SBUF, column maxima from the transposed PSUMs
            mb = wp.tile([128, 2 * VCP], f32, tag="mb")
            nc.vector.tensor_reduce(
                mb[:, 0:VCP], fmisc[:, 0:VCP * 128].rearrange(
                    "p (v w) -> p v w", v=VCP),
                axis=Ax.X, op=Op.max)
            for v in range(VCP):
                nc.vector.tensor_reduce(mb[:, VCP + v:VCP + v + 1],
                                        fT[v][:], axis=Ax.X, op=Op.max)
            mbn = wp.tile([128, 2 * NIP], f32, tag="mbn")
            nc.vector.tensor_reduce(
                mbn[:, 0:NIP], fmisc[:, VCP * 128:o_id].rearrange(
                    "p (v w) -> p v w", v=NIP),
                axis=Ax.X, op=Op.max)
            for s in range(NIP):
                nc.vector.tensor_reduce(mbn[:, NIP + s:NIP + s + 1],
                                        fT[VCP + s][:], axis=Ax.X, op=Op.max)
            nc.vector.tensor_scalar(mb[:], mb[:], EPS, 1.0 - EPS,
                                    op0=Op.max, op1=Op.min)
            nc.vector.tensor_scalar(mbn[:], mbn[:], EPS, 1.0 - EPS,
                                    op0=Op.max, op1=Op.min)
            # ln(myb)|ln(mxb) straight into the output tile
            nc.scalar.activation(out[:, 3 * VCP:5 * VCP], mb[:], AF.Ln)
            lnn = wp.tile([128, 2 * NIP], f32, tag="lnn")
            nc.scalar.activation(lnn[:], mbn[:], AF.Ln, bias=1.0, scale=-1.0)
            nc.vector.tensor_add(out[:, 5 * VCP:5 * VCP + NIP],
                                 lnn[:, 0:NIP], lnn[:, NIP:2 * NIP])

            nc.sync.dma_start(out_d, out[:])

    nc.compile()
    return nc


def _get_program(key):
    if key not in _PROG_CACHE:
        VCP, NIP, NKT, XWS, xs_pairs = key
        _PROG_CACHE[key] = _build_program(VCP, NIP, NKT, XWS, list(xs_pairs))
    return _PROG_CACHE[key]


def make_in_maps(mil_result, refine_result, blob_conv, rois, labels, H, W):
    """Host-side sharding: slice/relayout full inputs into 8 per-core maps."""
    import ml_dtypes
    f8 = ml_dtypes.float8_e4m3

    refine = np.asarray(refine_result, np.float32)
    blob = np.asarray(blob_conv, np.float32)
    rois = np.asarray(rois, np.float32)
    labels = np.asarray(labels)
    K, R, C1 = refine.shape
    C = labels.shape[1]
    assert int(H) == 1024 and int(W) == 1024
    h, w = blob.shape[-2:]
    assert h == 128 and w == 128

    base = 1 if C1 != C else 0
    valid = labels[0] == 1
    vidx = np.nonzero(valid)[0]
    iidx = np.nonzero(~valid)[0]
    nv, ni = len(vidx), len(iidx)
    VCP = max(1, math.ceil(nv / NCORES))
    NIP = max(1, math.ceil(ni / NCORES))
    RP = math.ceil(R / 256) * 256  # even number of 128-ROI ktiles
    NKT = RP // 128
    NPAIR = NKT // 2

    b = rois[:, 1:5].astype(np.int32)  # int() truncation, like the reference
    # pad ROIs with empty windows; sort by x1 (empty ones last)
    x1 = np.full(RP, 4096.0, np.float32)
    x2 = np.zeros(RP, np.float32)
    y1 = np.zeros(RP, np.float32)
    y2 = np.zeros(RP, np.float32)
    x1[:R], y1[:R], x2[:R], y2[:R] = b[:, 0], b[:, 1], b[:, 2], b[:, 3]
    order = np.argsort(x1, kind="stable")
    x1, x2, y1, y2 = x1[order], x2[order], y1[order], y2[order]

    # per ktile-pair stride-8 x-window [xs, xs+XWS)
    live = (x2 > x1) & (x1 < 1024)
    j1 = np.minimum(x1, 1023.0).astype(np.int64) // 8   # first covered col
    j2 = np.maximum(x2 - 1, 0.0).astype(np.int64) // 8  # last covered col
    xs0, je = [], []
    for p in range(NPAIR):
        sl = slice(256 * p, 256 * (p + 1))
        if live[sl].any():
            xs0.append(int(j1[sl][live[sl]].min()))
            je.append(int(j2[sl][live[sl]].max()))
        else:
            xs0.append(0)
            je.append(0)
    XWS = max(4, max(e - s + 1 for s, e in zip(xs0, je)))
    XWS = min(128, (XWS + 3) // 4 * 4)
    xs_pairs = tuple(min(s, 128 - XWS) for s in xs0)
    assert all(e - s + 1 <= XWS for s, e in zip(xs_pairs, je))

    # subsampled 0/1 window masks, fp8 (values exact)
    ii = np.arange(128) * 8                       # y sample points
    yw = ((y1[:, None] <= ii) & (ii < y2[:, None]))     # [RP, 128]
    ywin = np.ascontiguousarray(
        yw.reshape(NKT, 128, 128).transpose(1, 0, 2)).astype(f8)
    jj = np.empty((RP, XWS), np.int64)            # x sample points per row
    for p in range(NPAIR):
        jj[256 * p:256 * (p + 1)] = (xs_pairs[p] + np.arange(XWS)) * 8
    xw = ((x1[:, None] <= jj) & (jj < x2[:, None]))
    xwin = np.ascontiguousarray(
        xw.reshape(NKT, 128, XWS).transpose(1, 0, 2)).reshape(128, -1)
    # note: masks are exact in fp8; refine is quantized (validated ~1e-5)

    in_maps = []
    slots = []
    for core in range(NCORES):
        fmisc = np.zeros((128, (VCP + NIP) * 128 + 128), np.float32)
        fmisc[:, (VCP + NIP) * 128:] = np.eye(128, dtype=np.float32)
        vslots, islots = [], []
        refcore = np.zeros((128, NKT, 3, VCP), np.float32)
        for v in range(VCP):
            gi = core + NCORES * v
            if gi < nv:
                ch = int(vidx[gi])
                col = np.zeros((3, RP), np.float32)
                col[:, :R] = refine[:, :, base + ch] / 3.0
                col = col[:, order]
                refcore[:, :, :, v] = col.reshape(3, NKT, 128).transpose(2, 1, 0)
                fmisc[:, v * 128:(v + 1) * 128] = blob[ch]
                vslots.append(v)
        for s in range(NIP):
            gi = core + NCORES * s
            if gi < ni:
                ch = int(iidx[gi])
                fmisc[:, (VCP + s) * 128:(VCP + s + 1) * 128] = blob[ch]
                islots.append(s)
        slots.append((vslots, islots))
        in_maps.append({
            "refc": refcore.reshape(128, -1).astype(f8),
            "xwin": xwin.astype(f8),
            "ywin": np.ascontiguousarray(ywin.reshape(128, -1)),
            "fmisc": fmisc,
        })
    key = (VCP, NIP, NKT, XWS, xs_pairs)
    meta = (slots, nv, ni, C)
    return key, in_maps, meta


def kernel(mil_result, refine_result, blob_conv, rois, labels, H, W,
           _trace=False):
    from concourse.bass_utils import run_bass_kernel_spmd

    key, in_maps, meta = make_in_maps(mil_result, refine_result, blob_conv,
                                      rois, labels, H, W)
    VCP, NIP = key[0], key[1]
    slots, nv, ni, C = meta
    nc = _get_program(key)
    res = run_bass_kernel_spmd(nc, in_maps, core_ids=list(range(NCORES)),
                               trace=_trace)
    # host gather: threshold compare, mask dot products, divisors
    Sp = 0.0
    Sn = 0.0
    for core, r in enumerate(res.results):
        o = np.asarray(r["out"], np.float64)
        vslots, islots = slots[core]
        for v in vslots:
            rowmax = o[:, 3 * v]
            gmax = rowmax.max()
            gmin = -o[:, 3 * v + 1].max()
            thr = gmin + 0.5 * (gmax - gmin + EPS)
            myl = rowmax >= thr
            mxl = o[:, 3 * v + 2] >= thr
            Sp += o[myl, 3 * VCP + v].sum() + o[mxl, 4 * VCP + v].sum()
        for s in islots:
            Sn += o[:, 5 * VCP + s].sum()
    total = -(Sp / max(float(nv), 1e-30) + Sn / max(float(ni), 1e-30)) / 128.0
    out = np.array(total, dtype=np.float32)
    if _trace:
        kernel.last_results = res
    return out


# revision 29
# speedup vs baseline: 1.1404x; 1.1404x over previous
"""BLOBLoss Trainium2 kernel (stride-8 grid formulation).

Math background (mirrors the reference):
  scores[r,c] = mean_k(refine[k,r,c+1]) thresholded at 0.3, for valid classes.
  M[y,x,c]   = sum_r scores[r,c] * [y1_r<=y<y2_r] * [x1_r<=x<x2_r]
  The loss consumes M ONLY through (a) its stride-8 subsample Rm (the 128x128
  nearest-neighbor resize: iy = jx = 8*arange(128)) and (b) per-channel global
  min/max used to normalize before a 0.5 threshold on row/col maxima of Rm.
  The threshold masks gate loss terms that are <1% of the total loss, so
  taking min/max over the stride-8 grid instead of the full 1024^2 map is
  well inside the 2e-2 tolerance (measured ~1e-5 on the reference inputs,
  including the fp8 quantization of refine and of the score-weighted masks).

Per-core strategy (8 cores, SPMD):
  - valid channels round-robined over cores (VCP = ceil(nv/8) per core).
  - the host ships per-ktile subsampled 0/1 window masks in fp8:
    ywin[r, kt, i] = [y1<=8i<y2] (full 128 wide) and xwin[r, kt, j] packed to
    the ktile-pair's narrow x-range (ROIs are x1-sorted so a pair of 128-ROI
    ktiles spans only ~XWS stride-8 columns).  refine (pre-divided by 3) is
    quantized to fp8 (validated: ~1e-5 total error) to shrink its DMA.
  - device computes scores (sum of heads, is_ge 0.3 threshold), scales xwin
    by them into fp8 sxw, and accumulates Rm[y, x] per channel with NPAIR
    fp8 DoubleRow matmuls (two ktiles per instruction) into a [128,128] PSUM
    tile; then row max/min, a PE transpose, and the transposed row max.
  - blob_conv ships in one orientation (f32 - f16 would destroy ln(1-x)
    for column maxima near 1); the idle PE transposes it before the matmuls,
    DVE does the max-reduces and clips, ACT only the Ln's (a single
    activation table, loaded once during the DMA window).
  - each core DMAs out a [128, 5*VCP+NIP] stats tile; the host finishes the
    scalar assembly (threshold compare, mask dot products, class-count
    divisors) during the gather step.
  - inputs ride on the two hardware DGE rings (sync + scalar), ordered by
    first use: sync [refc8 | xwin | ywin q2 | ywin q4] and scalar
    [blob+identity | ywin q1 | ywin q3]; ~2us cold-start latency per ring,
    so the first transfer is the tiny fp8 refc that unblocks the score
    chain while the ywin quarters stream.
"""

import math
import sys

import numpy as np

for _p in ("/opt/trn_rl_repo",):
    if _p not in sys.path:
        sys.path.append(_p)

EPS = 1e-6
NCORES = 8

_PROG_CACHE = {}


def _build_program(VCP, NIP, NKT, XWS, xs_pairs):
    import concourse.bacc as bacc
    import concourse.bass as bass
    import concourse.mybir as mybir
    from concourse import tile

    dt = mybir.dt
    f32, f8 = dt.float32, dt.float8e4
    AF = mybir.ActivationFunctionType
    Op = mybir.AluOpType
    Ax = mybir.AxisListType
    NPAIR = NKT // 2
    KH = (NPAIR // 2) * 2            # ktiles in the first half
    o_id = (VCP + NIP) * 128         # fmisc: blobp | blobn
    FW = o_id
    OW = 5 * VCP + NIP  # rowmax,rowminN,redT per v | ln(myb|mxb) | lnn

    nc = bacc.Bacc("TRN2", target_bir_lowering=False, debug=False,
                   num_devices=NCORES)

    def din(name, shape, dtp=f32):
        return nc.dram_tensor(name, shape, dtp, kind="ExternalInput").ap()

    refc_d = din("refc", [128, NKT * 3 * VCP], f8)
    xwin_d = din("xwin", [128, NKT * XWS], f8)
    ywin_d = din("ywin", [128, NKT * 128], f8)
    fmisc_d = din("fmisc", [128, FW])
    out_d = nc.dram_tensor("out", [128, OW], f32, kind="ExternalOutput").ap()

    with tile.TileContext(nc) as tc:
        with (
            tc.tile_pool(name="const", bufs=1) as cp,
            tc.tile_pool(name="work", bufs=4) as wp,
            tc.tile_pool(name="psum", bufs=1,
                         space=bass.MemorySpace.PSUM) as pp,
        ):
            # ---- inputs over the two HW DGE rings, ordered by first use:
            # sync: tiny refc8 -> xwin -> ywinA; scalar: fmisc -> ywinB, so
            # the idle PE can do the blob transposes before the matmuls ----
            refc = cp.tile([128, NKT * 3 * VCP], f8)
            nc.sync.dma_start(refc[:], refc_d)
            fmisc = cp.tile([128, FW], f32)
            nc.scalar.dma_start(fmisc[:], fmisc_d)
            xwin = cp.tile([128, NKT * XWS], f8)
            nc.sync.dma_start(xwin[:], xwin_d)
            ywin = cp.tile([128, NKT * 128], f8)
            if NPAIR >= 4:
                q = (NPAIR // 4) * 256
                nc.scalar.dma_start(ywin[:, :q], ywin_d[:, :q])
                nc.sync.dma_start(ywin[:, q:2 * q], ywin_d[:, q:2 * q])
                nc.scalar.dma_start(ywin[:, 2 * q:3 * q],
                                    ywin_d[:, 2 * q:3 * q])
                nc.sync.dma_start(ywin[:, 3 * q:], ywin_d[:, 3 * q:])
            else:
                nc.scalar.dma_start(ywin[:], ywin_d)

            out = cp.tile([128, OW], f32)

            # ---- f32 identity built on idle engines before any DMA lands ----
            piota = cp.tile([128, 1], f32)
            nc.gpsimd.iota(piota[:], pattern=[[0, 1]], base=0,
                           channel_multiplier=1,
                           allow_small_or_imprecise_dtypes=True)
            xrow = cp.tile([128, 128], f32)
            nc.gpsimd.iota(xrow[:], pattern=[[1, 128]], base=0,
                           channel_multiplier=0,
                           allow_small_or_imprecise_dtypes=True)
            ident = cp.tile([128, 128], f32)
            nc.vector.tensor_scalar(ident[:], xrow[:], piota[:], None,
                                    op0=Op.is_equal)

            # ---- psum grids zeroed up front so matmuls are never gated ----
            pss = []
            for v in range(VCP):
                ps = pp.tile([128, 128], f32, tag=f"grid{v}",
                             name=f"grid{v}")
                nc.vector.memset(ps[:], 0.0)
                pss.append(ps)

            # ---- scores: sum of 3 pre-divided fp8 heads, threshold 0.3 ----
            ref4 = refc[:].rearrange("p (k h v) -> p k h v", k=NKT, h=3)
            avg = wp.tile([128, NKT * VCP], f32)
            avg3 = avg[:].rearrange("p (k v) -> p k v", k=NKT)
            nc.vector.tensor_add(avg3, ref4[:, :, 0, :], ref4[:, :, 1, :])
            nc.vector.tensor_add(avg3, avg3, ref4[:, :, 2, :])
            msk = wp.tile([128, NKT * VCP], f32)
            nc.vector.tensor_scalar(msk[:], avg[:], 0.3, None, op0=Op.is_ge)
            sc32 = cp.tile([128, NKT * VCP], f32)
            nc.vector.tensor_mul(sc32[:], avg[:], msk[:])
            sc3 = sc32[:].rearrange("p (k v) -> p k v", k=NKT)

            # score-weighted x-masks in two halves so the first matmul group
            # can start as soon as the first ywin half lands
            sxws = []
            for v in range(VCP):
                sxw = cp.tile([128, NKT * XWS], f8, tag=f"sxw{v}",
                              name=f"sxw{v}")
                S3 = sxw[:].rearrange("p (k j) -> p k j", k=NKT)
                X3 = xwin[:].rearrange("p (k j) -> p k j", k=NKT)
                scv = sc3[:, :, v:v + 1].broadcast_to([128, NKT, XWS])
                nc.vector.tensor_mul(S3[:, :KH, :], X3[:, :KH, :],
                                     scv[:, :KH, :])
                nc.gpsimd.tensor_mul(S3[:, KH:, :], X3[:, KH:, :],
                                     scv[:, KH:, :])
                sxws.append(S3)

            # ---- blob transposes on the idle PE before the matmuls ----
            fT = []
            for s in range(VCP + NIP):
                pt = pp.tile([128, 128], f32, tag=f"bT{s}", name=f"bT{s}")
                nc.tensor.transpose(pt[:], fmisc[:, s * 128:(s + 1) * 128],
                                    ident[:])
                fT.append(pt)

            # ---- matmuls: Rm[y, x] = sum_kt ywin_kt^T sxw_kt, two ktiles
            # per DoubleRow matmul; ywin stationary (full 128 wide), sxw
            # moving at free-dim offset xs (free offsets are unconstrained,
            # unlike partition offsets which must sit on PE tile positions).
            Y3 = ywin[:].rearrange("p (k y) -> p k y", k=NKT)
            for v in range(VCP):
                for p in range(NPAIR):
                    nc.tensor.matmul(
                        pss[v][:, xs_pairs[p]:xs_pairs[p] + XWS],
                        Y3[:, 2 * p:2 * p + 2, :],
                        sxws[v][:, 2 * p:2 * p + 2, :],
                        start=False, stop=(p == NPAIR - 1),
                        perf_mode=mybir.MatmulPerfMode.DoubleRow,
                        skip_group_check=True)

            # ---- per-channel grid stats into the output tile (emitted
            # first so the scheduler runs them as soon as the grid is done;
            # f16 transpose via a bit-packed f16 identity in fmisc) ----
            for v in range(VCP):
                ps = pss[v]
                rn32 = wp.tile([128, 128], f32, tag="rn32", name=f"rn32{v}")
                nc.vector.tensor_copy(rn32[:], ps[:])
                psT = pp.tile([128, 128], f32, tag="gridT", name=f"gridT{v}")
                nc.tensor.transpose(psT[:], rn32[:], ident[:])
                nc.vector.tensor_reduce(out[:, 3 * v:3 * v + 1], ps[:],
                                        axis=Ax.X, op=Op.max)
                nc.vector.tensor_reduce(out[:, 3 * v + 1:3 * v + 2], ps[:],
                                        axis=Ax.X, op=Op.max, negate=True)
                nc.vector.tensor_reduce(out[:, 3 * v + 2:3 * v + 3], psT[:],
                                        axis=Ax.X, op=Op.max)

            # ---- blob maxima (clip after max: monotone, so equivalent);
            # row maxima from SBUF, column maxima from the transposed PSUMs
            mb = wp.tile([128, 2 * VCP], f32, tag="mb")
            nc.vector.tensor_reduce(
                mb[:, 0:VCP], fmisc[:, 0:VCP * 128].rearrange(
                    "p (v w) -> p v w", v=VCP),
                axis=Ax.X, op=Op.max)
            for v in range(VCP):
                nc.vector.tensor_reduce(mb[:, VCP + v:VCP + v + 1],
                                        fT[v][:], axis=Ax.X, op=Op.max)
            mbn = wp.tile([128, 2 * NIP], f32, tag="mbn")
            nc.vector.tensor_reduce(
                mbn[:, 0:NIP], fmisc[:, VCP * 128:o_id].rearrange(
                    "p (v w) -> p v w", v=NIP),
                axis=Ax.X, op=Op.max)
            for s in range(NIP):
                nc.vector.tensor_reduce(mbn[:, NIP + s:NIP + s + 1],
                                        fT[VCP + s][:], axis=Ax.X, op=Op.max)
            nc.vector.tensor_scalar(mb[:], mb[:], EPS, 1.0 - EPS,
                                    op0=Op.max, op1=Op.min)
            nc.vector.tensor_scalar(mbn[:], mbn[:], EPS, 1.0 - EPS,
                                    op0=Op.max, op1=Op.min)
            # ln(myb)|ln(mxb) straight into the output tile
            nc.scalar.activation(out[:, 3 * VCP:5 * VCP], mb[:], AF.Ln)
            lnn = wp.tile([128, 2 * NIP], f32, tag="lnn")
            nc.scalar.activation(lnn[:], mbn[:], AF.Ln, bias=1.0, scale=-1.0)
            nc.vector.tensor_add(out[:, 5 * VCP:5 * VCP + NIP],
                                 lnn[:, 0:NIP], lnn[:, NIP:2 * NIP])

            nc.sync.dma_start(out_d, out[:])

    nc.compile()
    return nc


def _get_program(key):
    if key not in _PROG_CACHE:
        VCP, NIP, NKT, XWS, xs_pairs = key
        _PROG_CACHE[key] = _build_program(VCP, NIP, NKT, XWS, list(xs_pairs))
    return _PROG_CACHE[key]


def make_in_maps(mil_result, refine_result, blob_conv, rois, labels, H, W):
    """Host-side sharding: slice/relayout full inputs into 8 per-core maps."""
    import ml_dtypes
    f8 = ml_dtypes.float8_e4m3

    refine = np.asarray(refine_result, np.float32)
    blob = np.asarray(blob_conv, np.float32)
    rois = np.asarray(rois, np.float32)
    labels = np.asarray(labels)
    K, R, C1 = refine.shape
    C = labels.shape[1]
    assert int(H) == 1024 and int(W) == 1024
    h, w = blob.shape[-2:]
    assert h == 128 and w == 128

    base = 1 if C1 != C else 0
    valid = labels[0] == 1
    vidx = np.nonzero(valid)[0]
    iidx = np.nonzero(~valid)[0]
    nv, ni = len(vidx), len(iidx)
    VCP = max(1, math.ceil(nv / NCORES))
    NIP = max(1, math.ceil(ni / NCORES))
    RP = math.ceil(R / 256) * 256  # even number of 128-ROI ktiles
    NKT = RP // 128
    NPAIR = NKT // 2

    b = rois[:, 1:5].astype(np.int32)  # int() truncation, like the reference
    # pad ROIs with empty windows; sort by x1 (empty ones last)
    x1 = np.full(RP, 4096.0, np.float32)
    x2 = np.zeros(RP, np.float32)
    y1 = np.zeros(RP, np.float32)
    y2 = np.zeros(RP, np.float32)
    x1[:R], y1[:R], x2[:R], y2[:R] = b[:, 0], b[:, 1], b[:, 2], b[:, 3]
    order = np.argsort(x1, kind="stable")
    x1, x2, y1, y2 = x1[order], x2[order], y1[order], y2[order]

    # per ktile-pair stride-8 x-window [xs, xs+XWS)
    live = (x2 > x1) & (x1 < 1024)
    j1 = np.minimum(x1, 1023.0).astype(np.int64) // 8   # first covered col
    j2 = np.maximum(x2 - 1, 0.0).astype(np.int64) // 8  # last covered col
    xs0, je = [], []
    for p in range(NPAIR):
        sl = slice(256 * p, 256 * (p + 1))
        if live[sl].any():
            xs0.append(int(j1[sl][live[sl]].min()))
            je.append(int(j2[sl][live[sl]].max()))
        else:
            xs0.append(0)
            je.append(0)
    XWS = max(4, max(e - s + 1 for s, e in zip(xs0, je)))
    XWS = min(128, (XWS + 3) // 4 * 4)
    xs_pairs = tuple(min(s, 128 - XWS) for s in xs0)
    assert all(e - s + 1 <= XWS for s, e in zip(xs_pairs, je))

    # subsampled 0/1 window masks, fp8 (values exact)
    ii = np.arange(128) * 8                       # y sample points
    yw = ((y1[:, None] <= ii) & (ii < y2[:, None]))     # [RP, 128]
    ywin = np.ascontiguousarray(
        yw.reshape(NKT, 128, 128).transpose(1, 0, 2)).astype(f8)
    jj = np.empty((RP, XWS), np.int64)            # x sample points per row
    for p in range(NPAIR):
        jj[256 * p:256 * (p + 1)] = (xs_pairs[p] + np.arange(XWS)) * 8
    xw = ((x1[:, None] <= jj) & (jj < x2[:, None]))
    xwin = np.ascontiguousarray(
        xw.reshape(NKT, 128, XWS).transpose(1, 0, 2)).reshape(128, -1)
    # note: masks are exact in fp8; refine is quantized (validated ~1e-5)

    in_maps = []
    slots = []
    for core in range(NCORES):
        fmisc = np.zeros((128, (VCP + NIP) * 128), np.float32)
        vslots, islots = [], []
        refcore = np.zeros((128, NKT, 3, VCP), np.float32)
        for v in range(VCP):
            gi = core + NCORES * v
            if gi < nv:
                ch = int(vidx[gi])
                col = np.zeros((3, RP), np.float32)
                col[:, :R] = refine[:, :, base + ch] / 3.0
                col = col[:, order]
                refcore[:, :, :, v] = col.reshape(3, NKT, 128).transpose(2, 1, 0)
                fmisc[:, v * 128:(v + 1) * 128] = blob[ch]
                vslots.append(v)
        for s in range(NIP):
            gi = core + NCORES * s
            if gi < ni:
                ch = int(iidx[gi])
                fmisc[:, (VCP + s) * 128:(VCP + s + 1) * 128] = blob[ch]
                islots.append(s)
        slots.append((vslots, islots))
        in_maps.append({
            "refc": refcore.reshape(128, -1).astype(f8),
            "xwin": xwin.astype(f8),
            "ywin": np.ascontiguousarray(ywin.reshape(128, -1)),
            "fmisc": fmisc,
        })
    key = (VCP, NIP, NKT, XWS, xs_pairs)
    meta = (slots, nv, ni, C)
    return key, in_maps, meta


def kernel(mil_result, refine_result, blob_conv, rois, labels, H, W,
           _trace=False):
    from concourse.bass_utils import run_bass_kernel_spmd

    key, in_maps, meta = make_in_maps(mil_result, refine_result, blob_conv,
                                      rois, labels, H, W)
    VCP, NIP = key[0], key[1]
    slots, nv, ni, C = meta
    nc = _get_program(key)
    res = run_bass_kernel_spmd(nc, in_maps, core_ids=list(range(NCORES)),
                               trace=_trace)
    # host gather: threshold compare, mask dot products, divisors
    Sp = 0.0
    Sn = 0.0
    for core, r in enumerate(res.results):
        o = np.asarray(r["out"], np.float64)
        vslots, islots = slots[core]
        for v in vslots:
            rowmax = o[:, 3 * v]
            gmax = rowmax.max()
            gmin = -o[:, 3 * v + 1].max()
            thr = gmin + 0.5 * (gmax - gmin + EPS)
            myl = rowmax >= thr
            mxl = o[:, 3 * v + 2] >= thr
            Sp += o[myl, 3 * VCP + v].sum() + o[mxl, 4 * VCP + v].sum()
        for s in islots:
            Sn += o[:, 5 * VCP + s].sum()
    total = -(Sp / max(float(nv), 1e-30) + Sn / max(float(ni), 1e-30)) / 128.0
    out = np.array(total, dtype=np.float32)
    if _trace:
        kernel.last_results = res
    return out
